# revision 15
# baseline (speedup 1.0000x reference)
"""Trainium2 Bass kernel for nn_EqStftPBC (STFT perturbation-based compensation).

v3: j-split sharding (core c: n2 in {5c-20..5c-16}, all 4 (b,m) signals),
host sums the 8 partial deltas.

- Per-core base shift folded into STFT weights (Xs); residual rolls r=1..4
  as permutation matmuls on the otherwise-idle PE.
- 51-dense plane-major layouts: one DVE op computes two real-product planes,
  evictions are flat copies.
- P^(1/3) folded into frames (delta is cubic in x), 1/cov into G weights,
  overlap-add folded into split G weights (Ga/Gb -> two 40-row PSUM banks).
"""

import numpy as np
from ml_dtypes import bfloat16

import concourse.bass as bass
import concourse.bacc as bacc
import concourse.mybir as mybir
import concourse.tile as tile

F = 80
T = 51
TP = 52
HOP = 40
L = 2080
BM = 4            # (b, m) units, all on every core
NJ = 5            # n2 per core: n2 = 5*core - 20 + r
CD = BM * T       # 204: dense (bm, t) slot per (plane, j)
WD = NJ * CD      # 1020: one plane across all j
FP32 = mybir.dt.float32
BF16 = mybir.dt.bfloat16
CPY = mybir.ActivationFunctionType.Copy


def _ap(t_ap, off, dims):
    return bass.AP(tensor=t_ap.tensor, offset=t_ap.offset + off,
                   ap=[t_ap.ap[0]] + dims)


def build_program(debug=False):
    nc = bacc.Bacc("TRN2", target_bir_lowering=False, debug=debug)

    xf = nc.dram_tensor("xf", [F, 3 * CD], BF16, kind="ExternalInput")
    fw = nc.dram_tensor("fw", [F, 4 * F], BF16, kind="ExternalInput")
    pw = nc.dram_tensor("pw", [F, 4 * F], BF16, kind="ExternalInput")
    mw = nc.dram_tensor("mw", [F, NJ * 3 * F], BF16, kind="ExternalInput")
    gw = nc.dram_tensor("gw", [F, 6 * HOP], BF16, kind="ExternalInput")
    yv = nc.dram_tensor("yv", [HOP, 2 * BM * TP], BF16, kind="ExternalOutput")

    MUL = mybir.AluOpType.mult
    ADD = mybir.AluOpType.add
    SUB = mybir.AluOpType.subtract

    with tile.TileContext(nc) as tc:
        with (
            tc.tile_pool(name="const", bufs=1) as cpool,
            tc.tile_pool(name="work", bufs=1) as wpool,
            tc.tile_pool(name="ps_s", bufs=1, space="PSUM") as ps_s,
            tc.tile_pool(name="ps_u", bufs=3, space="PSUM") as ps_u,
        ):
            # ---- input DMAs spread across queues; STFT inputs first ----
            xfs = wpool.tile([F, 3 * CD], BF16, tag="xfs")
            HX = 3 * CD // 2
            nc.sync.dma_start(xfs[:, 0:HX], xf[:, 0:HX])
            nc.gpsimd.dma_start(xfs[:, HX:3 * CD], xf[:, HX:3 * CD])
            fws = cpool.tile([F, 4 * F], BF16, tag="fws")
            nc.scalar.dma_start(fws[:, 2 * F:4 * F], fw[:, 2 * F:4 * F])
            nc.scalar.dma_start(fws[:, 0:2 * F], fw[:, 0:2 * F])
            pws = cpool.tile([F, 4 * F], BF16, tag="pws")
            nc.scalar.dma_start(pws[:, :], pw[:, :])
            mws = cpool.tile([F, NJ * 3 * F], BF16, tag="mws")
            HM = NJ * 3 * F // 2
            nc.gpsimd.dma_start(mws[:, 0:HM], mw[:, 0:HM])
            nc.sync.dma_start(mws[:, HM:2 * HM], mw[:, HM:2 * HM])
            gws = cpool.tile([F, 6 * HOP], BF16, tag="gws")
            nc.gpsimd.dma_start(gws[:, :], gw[:, :])

            # ---- STFT (Xs first: slot0 gates the R matmuls) ----
            Xsp = ps_s.tile([F, 2 * CD], FP32, tag="Xsp")
            X0p = ps_s.tile([F, 2 * CD], FP32, tag="X0p")
            nc.tensor.matmul(Xsp[:, :], fws[:, 2 * F:3 * F], xfs[:, CD:3 * CD],
                             start=True, stop=False)
            nc.tensor.matmul(Xsp[:, :], fws[:, 3 * F:4 * F], xfs[:, 0:2 * CD],
                             start=False, stop=True)
            nc.tensor.matmul(X0p[:, :], fws[:, 0:F], xfs[:, CD:3 * CD],
                             start=True, stop=False)
            nc.tensor.matmul(X0p[:, :], fws[:, F:2 * F], xfs[:, 0:2 * CD],
                             start=False, stop=True)

            # Rall: plane-major [Rr(5j) | Ri(5j)], slot j = roll(Xs, j)
            Rall = wpool.tile([F, 2 * WD], BF16, tag="Rall")
            nc.scalar.activation(_ap(Rall[:, :], 0, [[WD, 2], [1, CD]]),
                                 Xsp[:, :], CPY)

            # X0T: [X0r x5 | X0i x5] tiled across j slots
            X0T = wpool.tile([F, 2 * WD], BF16, tag="X0T")
            nc.scalar.activation(_ap(X0T[:, :], 0, [[WD, 2], [1, CD]]),
                                 X0p[:, :], CPY)
            for pl in range(2):
                nc.scalar.activation(
                    _ap(X0T[:, :], pl * WD + CD, [[1, 4 * CD]]),
                    X0T[:, None, pl * WD:pl * WD + CD].to_broadcast(
                        [F, 4, CD]), CPY)

            # ---- residual rolls: r=1,2 permutation matmuls; r=3,4 DMA ----
            for r in (1, 2):
                Rp = ps_u.tile([F, 2 * CD], FP32, tag="Up")
                rhs = _ap(Rall[:, :], 0, [[WD, 2], [1, CD]])
                nc.tensor.matmul(Rp[:, :], pws[:, (r - 1) * F:r * F], rhs,
                                 start=True, stop=True)
                dst = _ap(Rall[:, :], r * CD, [[WD, 2], [1, CD]])
                src = _ap(Rp[:, :], 0, [[CD, 2], [1, CD]])
                nc.scalar.activation(dst, src, CPY)
            for r in (3, 4):
                # dst partitions r..79 <- src partitions 0..79-r (both planes)
                nc.sync.dma_start(
                    bass.AP(tensor=Rall[:, :].tensor,
                            offset=Rall[:, :].offset + r * 2 * WD + r * CD,
                            ap=[[2 * WD, F - r], [WD, 2], [1, CD]]),
                    bass.AP(tensor=Rall[:, :].tensor,
                            offset=Rall[:, :].offset,
                            ap=[[2 * WD, F - r], [WD, 2], [1, CD]]))
                nc.sync.dma_start(
                    bass.AP(tensor=Rall[:, :].tensor,
                            offset=Rall[:, :].offset + r * CD,
                            ap=[[2 * WD, r], [WD, 2], [1, CD]]),
                    bass.AP(tensor=Rall[:, :].tensor,
                            offset=Rall[:, :].offset + (F - r) * 2 * WD,
                            ap=[[2 * WD, r], [WD, 2], [1, CD]]))

            # ---- C stage (grouped) ----
            CS = wpool.tile([F, 4 * WD], BF16, tag="CS")   # [sA5|sC5|sB5|sD5]
            Cp = wpool.tile([F, 2 * WD], BF16, tag="Cp")
            Call = wpool.tile([F, 2 * WD], BF16, tag="Call")
            TTv = nc.vector.tensor_tensor
            TTg = nc.gpsimd.tensor_tensor

            def c_group(j0, nj):
                o = j0 * CD
                n = nj * CD
                u = nj * BM
                # [sA|sC] = [X0r|X0i] (x) Rr ; [sB|sD] = [X0i|X0r] (x) Ri
                TTv(_ap(CS[:, :], o, [[WD, 2], [1, n]]),
                    _ap(X0T[:, :], o, [[WD, 2], [1, n]]),
                    _ap(Rall[:, :], o, [[0, 2], [1, n]]), MUL)
                TTv(_ap(CS[:, :], 2 * WD + o, [[WD, 2], [1, n]]),
                    _ap(X0T[:, :], WD + o, [[-WD, 2], [1, n]]),
                    _ap(Rall[:, :], WD + o, [[0, 2], [1, n]]), MUL)
                # Crp = sA+sB ; Cip = sC-sD   (flat, into Cp planes)
                TTv(_ap(Cp[:, :], o, [[1, n]]),
                    _ap(CS[:, :], o, [[1, n]]),
                    _ap(CS[:, :], 2 * WD + o, [[1, n]]), ADD)
                TTv(_ap(Cp[:, :], WD + o, [[1, n]]),
                    _ap(CS[:, :], WD + o, [[1, n]]),
                    _ap(CS[:, :], 3 * WD + o, [[1, n]]), SUB)
                # roll-add over t within each (j,bm) block of 51
                TTv(_ap(Call[:, :], o + 1, [[T, u], [1, T - 1]]),
                    _ap(Cp[:, :], o + 1, [[T, u], [1, T - 1]]),
                    _ap(Cp[:, :], o, [[T, u], [1, T - 1]]), ADD)
                TTg(_ap(Call[:, :], WD + o + 1, [[T, u], [1, T - 1]]),
                    _ap(Cp[:, :], WD + o + 1, [[T, u], [1, T - 1]]),
                    _ap(Cp[:, :], WD + o, [[T, u], [1, T - 1]]), ADD)
                TTv(_ap(Call[:, :], o, [[T, u]]),
                    _ap(Cp[:, :], o, [[T, u]]),
                    _ap(Cp[:, :], o + T - 1, [[T, u]]), ADD)
                TTg(_ap(Call[:, :], WD + o, [[T, u]]),
                    _ap(Cp[:, :], WD + o, [[T, u]]),
                    _ap(Cp[:, :], WD + o + T - 1, [[T, u]]), ADD)

            # ---- per-j stages ----
            VS = wpool.tile([F, 8 * CD], BF16, tag="VS")
            Vall = wpool.tile([F, 2 * WD], BF16, tag="Vall")
            Ya = ps_s.tile([HOP, 2 * CD], FP32, tag="Ya")
            Yb = ps_s.tile([HOP, 2 * CD], FP32, tag="Yb")
            Ups = [None] * NJ

            def u_mm(j):
                Up = ps_u.tile([F, 2 * CD], FP32, tag="Up")
                Ups[j] = Up
                rhs2 = _ap(Call[:, :], j * CD, [[WD, 2], [1, CD]])
                rhs_i = _ap(Call[:, :], WD + j * CD, [[1, CD]])
                rhs_r = _ap(Call[:, :], j * CD, [[1, CD]])
                mo = j * 3 * F
                nc.tensor.matmul(Up[:, :], mws[:, mo:mo + F], rhs2,
                                 start=True, stop=False)
                nc.tensor.matmul(Up[:, 0:CD], mws[:, mo + F:mo + 2 * F], rhs_i,
                                 start=False, stop=False)
                nc.tensor.matmul(Up[:, CD:2 * CD], mws[:, mo + 2 * F:mo + 3 * F],
                                 rhs_r, start=False, stop=True)

            def v_tt(j):
                p = (j % 2) * 4 * CD
                uu = Ups[j][:, :]   # [Ur|Ui] read directly from PSUM
                # [tA|tB] = [Ur|Ui] (x) [Rr|Ri] ; [tC|tD] = [Ur|Ui] (x) [Ri|Rr]
                TTv(_ap(VS[:, :], p, [[1, 2 * CD]]), uu,
                    _ap(Rall[:, :], j * CD, [[WD, 2], [1, CD]]), MUL)
                TTv(_ap(VS[:, :], p + 2 * CD, [[1, 2 * CD]]), uu,
                    _ap(Rall[:, :], WD + j * CD, [[-WD, 2], [1, CD]]), MUL)
                # Vr = tA - tB ; Vi = tC + tD
                TTv(_ap(Vall[:, :], j * 2 * CD, [[1, CD]]),
                    _ap(VS[:, :], p, [[1, CD]]),
                    _ap(VS[:, :], p + CD, [[1, CD]]), SUB)
                TTg(_ap(Vall[:, :], j * 2 * CD + CD, [[1, CD]]),
                    _ap(VS[:, :], p + 2 * CD, [[1, CD]]),
                    _ap(VS[:, :], p + 3 * CD, [[1, CD]]), ADD)

            def d_mm(j):
                rhs2 = _ap(Vall[:, :], j * 2 * CD, [[CD, 2], [1, CD]])
                rhs_i = _ap(Vall[:, :], j * 2 * CD + CD, [[1, CD]])
                rhs_r = _ap(Vall[:, :], j * 2 * CD, [[1, CD]])
                st = (j == 0)
                sp = (j == NJ - 1)
                nc.tensor.matmul(Ya[:, :], gws[:, 0:HOP], rhs2,
                                 start=st, stop=False)
                nc.tensor.matmul(Yb[:, :], gws[:, HOP:2 * HOP], rhs2,
                                 start=st, stop=False)
                nc.tensor.matmul(Ya[:, 0:CD], gws[:, 2 * HOP:3 * HOP], rhs_i,
                                 start=False, stop=False)
                nc.tensor.matmul(Yb[:, 0:CD], gws[:, 3 * HOP:4 * HOP], rhs_i,
                                 start=False, stop=False)
                nc.tensor.matmul(Ya[:, CD:2 * CD], gws[:, 4 * HOP:5 * HOP],
                                 rhs_r, start=False, stop=sp)
                nc.tensor.matmul(Yb[:, CD:2 * CD], gws[:, 5 * HOP:6 * HOP],
                                 rhs_r, start=False, stop=sp)

            # ---- software-pipelined emission ----
            c_group(0, 2)
            u_mm(0)
            u_mm(1)
            c_group(2, 3)
            v_tt(0)
            u_mm(2)
            d_mm(0)
            v_tt(1)
            u_mm(3)
            d_mm(1)
            v_tt(2)
            u_mm(4)
            d_mm(2)
            v_tt(3)
            d_mm(3)
            v_tt(4)
            d_mm(4)

            # ---- tail: Y[tp] = Ya[t=tp] + Yb[t=tp-1], edges x2 ----
            Ysb = wpool.tile([HOP, 2 * BM * TP], BF16, tag="Ysb")
            Ybs = wpool.tile([HOP, 2 * CD], BF16, tag="Ybs")
            nc.scalar.activation(Ybs[:, :], Yb[:, :], CPY)
            CW = BM * TP
            for c2 in range(2):
                TTv(_ap(Ysb[:, :], c2 * CW + 1, [[TP, BM], [1, T - 1]]),
                    _ap(Ya[:, :], c2 * CD + 1, [[T, BM], [1, T - 1]]),
                    _ap(Ybs[:, :], c2 * CD, [[T, BM], [1, T - 1]]), ADD)
            nc.scalar.activation(
                _ap(Ysb[:, :], 0, [[CW, 2], [TP, BM]]),
                _ap(Ya[:, :], 0, [[CD, 2], [T, BM]]), CPY, scale=2.0)
            nc.scalar.activation(
                _ap(Ysb[:, :], T, [[CW, 2], [TP, BM]]),
                _ap(Ybs[:, :], T - 1, [[CD, 2], [T, BM]]), CPY, scale=2.0)
            nc.sync.dma_start(yv[:, 0:CW], Ysb[:, 0:CW])
            nc.scalar.dma_start(yv[:, CW:2 * CW], Ysb[:, CW:2 * CW])
    return nc


# ---------------- host side ----------------

def _dft_consts():
    j = np.arange(F)
    W = np.exp(-2j * np.pi * np.outer(j, j) / F)
    G = np.exp(+2j * np.pi * np.outer(j, j) / F) / F
    return W, G


def _frame(sig):
    idx = np.arange(T)[None, :] * HOP + np.arange(F)[:, None]   # [g, t]
    return sig[idx].astype(np.float32)


def _m_mats(w2, n2):
    g = np.arange(F)[:, None]
    f = np.arange(F)[None, :]
    n1 = ((f - g + 20) % F) - 20
    valid = (n1 >= -20) & (n1 <= 19)
    n1c = np.clip(n1 + 20, 0, 39)
    col = w2[:, n2 + 20]
    Mr = np.where(valid, col.real[n1c], 0.0).astype(np.float32)
    Mi = np.where(valid, col.imag[n1c], 0.0).astype(np.float32)
    return Mr, Mi


def make_in_maps(x_real, x_imag, task_info, w_real, w_imag):
    W, G = _dft_consts()
    b, _, m = x_real.shape
    P = np.power(10.0, task_info[:, 0] / 10.0) / m
    w2 = (np.asarray(w_real) + 1j * np.asarray(w_imag)).reshape(40, 40)

    frs, fis = [], []
    for bb in range(b):
        s = float(P[bb]) ** (1.0 / 3.0)
        for mm in range(m):
            frs.append(_frame(x_real[bb, :, mm]) * s)
            fis.append(_frame(x_imag[bb, :, mm]) * s)
    fr = np.stack(frs, 1)
    fi = np.stack(fis, 1)
    xfv = np.concatenate([(-fi).reshape(F, -1), fr.reshape(F, -1),
                          fi.reshape(F, -1)], axis=1).astype(bfloat16)

    # G folded: 1/cov=1/2, rows split [0:40)/[40:80) for fused overlap-add
    Gh = G * 0.5
    gwv = np.concatenate([Gh.real[0:HOP].T, Gh.real[HOP:F].T,
                          -Gh.imag[0:HOP].T, -Gh.imag[HOP:F].T,
                          Gh.imag[0:HOP].T, Gh.imag[HOP:F].T],
                         axis=1).astype(bfloat16)

    # permutation matrices for rolls r=1..4 (lhsT[g, f] = 1 iff g=(f-r)%80)
    pparts = []
    g = np.arange(F)
    for r in range(1, NJ):
        Pm = np.zeros((F, F), np.float32)
        Pm[(g - r) % F, g] = 1.0
        pparts.append(Pm)
    pwv = np.concatenate(pparts, axis=1).astype(bfloat16)

    in_maps, shards = [], []
    for ci in range(8):
        sc = 5 * ci - 20
        Ws = np.roll(W, sc, axis=0).T
        fwv = np.concatenate([W.real, W.imag, Ws.real, Ws.imag],
                             axis=1).astype(bfloat16)
        mparts = []
        for r in range(NJ):
            Mr, Mi = _m_mats(w2, sc + r)
            mparts += [Mr, -Mi, Mi]
        mwv = np.concatenate(mparts, axis=1).astype(bfloat16)
        in_maps.append({"xf": xfv, "fw": fwv, "pw": pwv, "mw": mwv,
                        "gw": gwv})
        shards.append(ci)

    cov = np.zeros(L)
    idx = (np.arange(T)[:, None] * HOP + np.arange(F)[None, :]).reshape(-1)
    np.add.at(cov, idx, 1.0)
    cov = np.where(cov > 0, cov, 1.0)
    return in_maps, shards, P, cov


_NC_CACHE = {}


def kernel(x_real, x_imag, task_info, w_real, w_imag, b_real, b_imag):
    x_real = np.asarray(x_real)
    x_imag = np.asarray(x_imag)
    task_info = np.asarray(task_info)
    b, Lx, m = x_real.shape
    assert (b, Lx, m) == (2, L, 2)

    if "nc" not in _NC_CACHE:
        nc_ = build_program(debug=False)
        nc_.compile()
        _NC_CACHE["nc"] = nc_
    nc = _NC_CACHE["nc"]

    in_maps, shards, P, cov = make_in_maps(x_real, x_imag, task_info,
                                           w_real, w_imag)
    from concourse.bass_utils import run_bass_kernel_spmd
    res = run_bass_kernel_spmd(nc, in_maps, list(range(8))).results

    CW = BM * TP
    Ysum = np.zeros((HOP, 2 * CW), np.float64)
    for i in range(8):
        Ysum += np.asarray(res[i]["yv"], np.float64)
    Y = Ysum.reshape(HOP, 2, BM, TP)

    x = (x_real + 1j * x_imag).astype(np.complex64)
    out = x.copy()
    bias = complex(np.asarray(b_real)[0], np.asarray(b_imag)[0])
    bias_sig = np.zeros(L, np.complex64)
    bias_sig[np.arange(T) * HOP] = bias
    bias_sig /= cov
    for u in range(BM):
        bb, mm = divmod(u, m)
        yr = Y[:, 0, u].T.ravel()[:L]
        yi = Y[:, 1, u].T.ravel()[:L]
        out[bb, :, mm] += (yr + 1j * yi).astype(np.complex64)
        out[bb, :, mm] += (P[bb] * bias_sig).astype(np.complex64)
    return out[:, 20:L - 20, :]


# revision 21
# speedup vs baseline: 1.0150x; 1.0150x over previous
"""Trainium2 Bass kernel for nn_EqStftPBC (STFT perturbation-based compensation).

v3: j-split sharding (core c: n2 in {5c-20..5c-16}, all 4 (b,m) signals),
host sums the 8 partial deltas.

- Per-core base shift folded into STFT weights (Xs); residual rolls r=1..4
  as permutation matmuls on the otherwise-idle PE.
- 51-dense plane-major layouts: one DVE op computes two real-product planes,
  evictions are flat copies.
- P^(1/3) folded into frames (delta is cubic in x), 1/cov into G weights,
  overlap-add folded into split G weights (Ga/Gb -> two 40-row PSUM banks).
"""

import numpy as np
from ml_dtypes import bfloat16

import concourse.bass as bass
import concourse.bacc as bacc
import concourse.mybir as mybir
import concourse.tile as tile

F = 80
T = 51
TP = 52
HOP = 40
L = 2080
BM = 4            # (b, m) units, all on every core
NJ = 5            # n2 per core: n2 = 5*core - 20 + r
CD = BM * T       # 204: dense (bm, t) slot per (plane, j)
WD = NJ * CD      # 1020: one plane across all j
FP32 = mybir.dt.float32
BF16 = mybir.dt.bfloat16
CPY = mybir.ActivationFunctionType.Copy


def _ap(t_ap, off, dims):
    return bass.AP(tensor=t_ap.tensor, offset=t_ap.offset + off,
                   ap=[t_ap.ap[0]] + dims)


def build_program(debug=False):
    nc = bacc.Bacc("TRN2", target_bir_lowering=False, debug=debug)

    xf = nc.dram_tensor("xf", [F, 3 * CD], BF16, kind="ExternalInput")
    fw = nc.dram_tensor("fw", [F, 4 * F], BF16, kind="ExternalInput")
    pw = nc.dram_tensor("pw", [F, 4 * F], BF16, kind="ExternalInput")
    mw = nc.dram_tensor("mw", [F, NJ * 3 * F], BF16, kind="ExternalInput")
    gw = nc.dram_tensor("gw", [F, 6 * HOP], BF16, kind="ExternalInput")
    yv = nc.dram_tensor("yv", [HOP, 2 * BM * TP], BF16, kind="ExternalOutput")

    MUL = mybir.AluOpType.mult
    ADD = mybir.AluOpType.add
    SUB = mybir.AluOpType.subtract

    with tile.TileContext(nc) as tc:
        with (
            tc.tile_pool(name="const", bufs=1) as cpool,
            tc.tile_pool(name="work", bufs=1) as wpool,
            tc.tile_pool(name="ps_s", bufs=1, space="PSUM") as ps_s,
            tc.tile_pool(name="ps_u", bufs=6, space="PSUM") as ps_u,
        ):
            # ---- input DMAs spread across queues; STFT inputs first ----
            xfs = wpool.tile([F, 3 * CD], BF16, tag="xfs")
            HX = 3 * CD // 2
            nc.sync.dma_start(xfs[:, 0:HX], xf[:, 0:HX])
            nc.gpsimd.dma_start(xfs[:, HX:3 * CD], xf[:, HX:3 * CD])
            fws = cpool.tile([F, 4 * F], BF16, tag="fws")
            nc.scalar.dma_start(fws[:, 2 * F:4 * F], fw[:, 2 * F:4 * F])
            nc.scalar.dma_start(fws[:, 0:2 * F], fw[:, 0:2 * F])
            pws = cpool.tile([F, 4 * F], BF16, tag="pws")
            nc.scalar.dma_start(pws[:, :], pw[:, :])
            mws = cpool.tile([F, NJ * 3 * F], BF16, tag="mws")
            HM = NJ * 3 * F // 2
            nc.gpsimd.dma_start(mws[:, 0:HM], mw[:, 0:HM])
            nc.sync.dma_start(mws[:, HM:2 * HM], mw[:, HM:2 * HM])
            gws = cpool.tile([F, 6 * HOP], BF16, tag="gws")
            nc.gpsimd.dma_start(gws[:, :], gw[:, :])

            # ---- STFT (Xs first: slot0 gates the R matmuls) ----
            Xsp = ps_u.tile([F, 2 * CD], FP32, tag="Up")
            X0p = ps_u.tile([F, 2 * CD], FP32, tag="Up")
            nc.tensor.matmul(Xsp[:, :], fws[:, 2 * F:3 * F], xfs[:, CD:3 * CD],
                             start=True, stop=False)
            nc.tensor.matmul(Xsp[:, :], fws[:, 3 * F:4 * F], xfs[:, 0:2 * CD],
                             start=False, stop=True)
            nc.tensor.matmul(X0p[:, :], fws[:, 0:F], xfs[:, CD:3 * CD],
                             start=True, stop=False)
            nc.tensor.matmul(X0p[:, :], fws[:, F:2 * F], xfs[:, 0:2 * CD],
                             start=False, stop=True)

            # Rall: plane-major [Rr(5j) | Ri(5j)], slot j = roll(Xs, j)
            Rall = wpool.tile([F, 2 * WD], BF16, tag="Rall")
            nc.scalar.activation(_ap(Rall[:, :], 0, [[WD, 2], [1, CD]]),
                                 Xsp[:, :], CPY)

            # X0T: [X0r x5 | X0i x5] tiled across j slots
            X0T = wpool.tile([F, 2 * WD], BF16, tag="X0T")
            nc.scalar.activation(_ap(X0T[:, :], 0, [[WD, 2], [1, CD]]),
                                 X0p[:, :], CPY)
            for pl in range(2):
                nc.vector.tensor_copy(
                    _ap(X0T[:, :], pl * WD + CD, [[1, 4 * CD]]),
                    X0T[:, None, pl * WD:pl * WD + CD].to_broadcast(
                        [F, 4, CD]))

            # ---- residual rolls: r=1,2 permutation matmuls; r=3,4 DMA ----
            for r in (1, 2):
                Rp = ps_u.tile([F, 2 * CD], FP32, tag="Up")
                rhs = _ap(Rall[:, :], 0, [[WD, 2], [1, CD]])
                nc.tensor.matmul(Rp[:, :], pws[:, (r - 1) * F:r * F], rhs,
                                 start=True, stop=True)
                dst = _ap(Rall[:, :], r * CD, [[WD, 2], [1, CD]])
                src = _ap(Rp[:, :], 0, [[CD, 2], [1, CD]])
                nc.scalar.activation(dst, src, CPY)
            for r in (3, 4):
                # dst partitions r..79 <- src partitions 0..79-r (both planes)
                nc.sync.dma_start(
                    bass.AP(tensor=Rall[:, :].tensor,
                            offset=Rall[:, :].offset + r * 2 * WD + r * CD,
                            ap=[[2 * WD, F - r], [WD, 2], [1, CD]]),
                    bass.AP(tensor=Rall[:, :].tensor,
                            offset=Rall[:, :].offset,
                            ap=[[2 * WD, F - r], [WD, 2], [1, CD]]))
                nc.sync.dma_start(
                    bass.AP(tensor=Rall[:, :].tensor,
                            offset=Rall[:, :].offset + r * CD,
                            ap=[[2 * WD, r], [WD, 2], [1, CD]]),
                    bass.AP(tensor=Rall[:, :].tensor,
                            offset=Rall[:, :].offset + (F - r) * 2 * WD,
                            ap=[[2 * WD, r], [WD, 2], [1, CD]]))

            # ---- C stage (grouped) ----
            CS = wpool.tile([F, 4 * WD], BF16, tag="CS")   # [sA5|sC5|sB5|sD5]
            Cp = wpool.tile([F, 2 * WD], BF16, tag="Cp")
            Call = wpool.tile([F, 2 * WD], BF16, tag="Call")
            TTv = nc.vector.tensor_tensor
            TTg = nc.gpsimd.tensor_tensor

            def c_group(j0, nj):
                o = j0 * CD
                n = nj * CD
                u = nj * BM
                # [sA|sC] = [X0r|X0i] (x) Rr ; [sB|sD] = [X0i|X0r] (x) Ri
                TTv(_ap(CS[:, :], o, [[WD, 2], [1, n]]),
                    _ap(X0T[:, :], o, [[WD, 2], [1, n]]),
                    _ap(Rall[:, :], o, [[0, 2], [1, n]]), MUL)
                TTv(_ap(CS[:, :], 2 * WD + o, [[WD, 2], [1, n]]),
                    _ap(X0T[:, :], WD + o, [[-WD, 2], [1, n]]),
                    _ap(Rall[:, :], WD + o, [[0, 2], [1, n]]), MUL)
                # Crp = sA+sB ; Cip = sC-sD   (flat, into Cp planes)
                TTv(_ap(Cp[:, :], o, [[1, n]]),
                    _ap(CS[:, :], o, [[1, n]]),
                    _ap(CS[:, :], 2 * WD + o, [[1, n]]), ADD)
                TTv(_ap(Cp[:, :], WD + o, [[1, n]]),
                    _ap(CS[:, :], WD + o, [[1, n]]),
                    _ap(CS[:, :], 3 * WD + o, [[1, n]]), SUB)
                # roll-add over t within each (j,bm) block of 51
                TTv(_ap(Call[:, :], o + 1, [[T, u], [1, T - 1]]),
                    _ap(Cp[:, :], o + 1, [[T, u], [1, T - 1]]),
                    _ap(Cp[:, :], o, [[T, u], [1, T - 1]]), ADD)
                TTg(_ap(Call[:, :], WD + o + 1, [[T, u], [1, T - 1]]),
                    _ap(Cp[:, :], WD + o + 1, [[T, u], [1, T - 1]]),
                    _ap(Cp[:, :], WD + o, [[T, u], [1, T - 1]]), ADD)
                TTv(_ap(Call[:, :], o, [[T, u]]),
                    _ap(Cp[:, :], o, [[T, u]]),
                    _ap(Cp[:, :], o + T - 1, [[T, u]]), ADD)
                TTg(_ap(Call[:, :], WD + o, [[T, u]]),
                    _ap(Cp[:, :], WD + o, [[T, u]]),
                    _ap(Cp[:, :], WD + o + T - 1, [[T, u]]), ADD)

            # ---- per-j stages ----
            VS = wpool.tile([F, 8 * CD], BF16, tag="VS")
            Vall = wpool.tile([F, 2 * WD], BF16, tag="Vall")
            VQ = wpool.tile([F, 8 * CD], BF16, tag="VQ")  # S01|S23|S0123|VS5
            Ya = ps_s.tile([HOP, 2 * CD], FP32, tag="Ya")
            Yb = ps_s.tile([HOP, 2 * CD], FP32, tag="Yb")
            Ups = [None] * NJ

            def u_mm(j):
                Up = ps_u.tile([F, 2 * CD], FP32, tag="Up")
                Ups[j] = Up
                rhs2 = _ap(Call[:, :], j * CD, [[WD, 2], [1, CD]])
                rhs_i = _ap(Call[:, :], WD + j * CD, [[1, CD]])
                rhs_r = _ap(Call[:, :], j * CD, [[1, CD]])
                mo = j * 3 * F
                nc.tensor.matmul(Up[:, :], mws[:, mo:mo + F], rhs2,
                                 start=True, stop=False)
                nc.tensor.matmul(Up[:, 0:CD], mws[:, mo + F:mo + 2 * F], rhs_i,
                                 start=False, stop=False)
                nc.tensor.matmul(Up[:, CD:2 * CD], mws[:, mo + 2 * F:mo + 3 * F],
                                 rhs_r, start=False, stop=True)

            def v_tt(j):
                p = (j % 2) * 4 * CD
                uu = Ups[j][:, :]   # [Ur|Ui] read directly from PSUM
                # [tA|tB] = [Ur|Ui] (x) [Rr|Ri] ; [tC|tD] = [Ur|Ui] (x) [Ri|Rr]
                TTv(_ap(VS[:, :], p, [[1, 2 * CD]]), uu,
                    _ap(Rall[:, :], j * CD, [[WD, 2], [1, CD]]), MUL)
                TTv(_ap(VS[:, :], p + 2 * CD, [[1, 2 * CD]]), uu,
                    _ap(Rall[:, :], WD + j * CD, [[-WD, 2], [1, CD]]), MUL)
                # Vr = tA - tB ; Vi = tC + tD
                TTv(_ap(Vall[:, :], j * 2 * CD, [[1, CD]]),
                    _ap(VS[:, :], p, [[1, CD]]),
                    _ap(VS[:, :], p + CD, [[1, CD]]), SUB)
                TTg(_ap(Vall[:, :], j * 2 * CD + CD, [[1, CD]]),
                    _ap(VS[:, :], p + 2 * CD, [[1, CD]]),
                    _ap(VS[:, :], p + 3 * CD, [[1, CD]]), ADD)

            def v_sum(dst_o, a_ap, b_ap, eng):
                (TTv if eng == 'v' else TTg)(
                    _ap(VQ[:, :], dst_o, [[1, 2 * CD]]), a_ap, b_ap, ADD)

            # ---- software-pipelined emission ----
            c_group(0, 2)
            u_mm(0)
            u_mm(1)
            c_group(2, 3)
            v_tt(0)
            u_mm(2)
            v_tt(1)
            u_mm(3)
            v_tt(2)
            v_sum(0, Vall[:, 0:2 * CD], Vall[:, 2 * CD:4 * CD], 'g')
            u_mm(4)
            v_tt(3)
            v_tt(4)
            v_sum(2 * CD, Vall[:, 4 * CD:6 * CD], Vall[:, 6 * CD:8 * CD], 'g')
            v_sum(4 * CD, VQ[:, 0:2 * CD], VQ[:, 2 * CD:4 * CD], 'v')
            v_sum(6 * CD, VQ[:, 4 * CD:6 * CD], Vall[:, 8 * CD:10 * CD], 'v')

            # ---- single D pass on Vsum ----
            rhs2 = _ap(VQ[:, :], 6 * CD, [[CD, 2], [1, CD]])
            rhs_i = _ap(VQ[:, :], 7 * CD, [[1, CD]])
            rhs_r = _ap(VQ[:, :], 6 * CD, [[1, CD]])
            nc.tensor.matmul(Ya[:, :], gws[:, 0:HOP], rhs2,
                             start=True, stop=False)
            nc.tensor.matmul(Yb[:, :], gws[:, HOP:2 * HOP], rhs2,
                             start=True, stop=False)
            nc.tensor.matmul(Ya[:, 0:CD], gws[:, 2 * HOP:3 * HOP], rhs_i,
                             start=False, stop=False)
            nc.tensor.matmul(Yb[:, 0:CD], gws[:, 3 * HOP:4 * HOP], rhs_i,
                             start=False, stop=False)
            nc.tensor.matmul(Ya[:, CD:2 * CD], gws[:, 4 * HOP:5 * HOP],
                             rhs_r, start=False, stop=True)
            nc.tensor.matmul(Yb[:, CD:2 * CD], gws[:, 5 * HOP:6 * HOP],
                             rhs_r, start=False, stop=True)

            # ---- tail: Y[tp] = Ya[t=tp] + Yb[t=tp-1], edges x2 ----
            Ysb = wpool.tile([HOP, 2 * BM * TP], BF16, tag="Ysb")
            Ybs = wpool.tile([HOP, 2 * CD], BF16, tag="Ybs")
            nc.scalar.activation(Ybs[:, :], Yb[:, :], CPY)
            CW = BM * TP
            for c2 in range(2):
                TTv(_ap(Ysb[:, :], c2 * CW + 1, [[TP, BM], [1, T - 1]]),
                    _ap(Ya[:, :], c2 * CD + 1, [[T, BM], [1, T - 1]]),
                    _ap(Ybs[:, :], c2 * CD, [[T, BM], [1, T - 1]]), ADD)
            nc.scalar.activation(
                _ap(Ysb[:, :], 0, [[CW, 2], [TP, BM]]),
                _ap(Ya[:, :], 0, [[CD, 2], [T, BM]]), CPY, scale=2.0)
            nc.scalar.activation(
                _ap(Ysb[:, :], T, [[CW, 2], [TP, BM]]),
                _ap(Ybs[:, :], T - 1, [[CD, 2], [T, BM]]), CPY, scale=2.0)
            nc.sync.dma_start(yv[:, 0:CW], Ysb[:, 0:CW])
            nc.scalar.dma_start(yv[:, CW:2 * CW], Ysb[:, CW:2 * CW])
    return nc


# ---------------- host side ----------------

def _dft_consts():
    j = np.arange(F)
    W = np.exp(-2j * np.pi * np.outer(j, j) / F)
    G = np.exp(+2j * np.pi * np.outer(j, j) / F) / F
    return W, G


def _frame(sig):
    idx = np.arange(T)[None, :] * HOP + np.arange(F)[:, None]   # [g, t]
    return sig[idx].astype(np.float32)


def _m_mats(w2, n2):
    g = np.arange(F)[:, None]
    f = np.arange(F)[None, :]
    n1 = ((f - g + 20) % F) - 20
    valid = (n1 >= -20) & (n1 <= 19)
    n1c = np.clip(n1 + 20, 0, 39)
    col = w2[:, n2 + 20]
    Mr = np.where(valid, col.real[n1c], 0.0).astype(np.float32)
    Mi = np.where(valid, col.imag[n1c], 0.0).astype(np.float32)
    return Mr, Mi


def make_in_maps(x_real, x_imag, task_info, w_real, w_imag):
    W, G = _dft_consts()
    b, _, m = x_real.shape
    P = np.power(10.0, task_info[:, 0] / 10.0) / m
    w2 = (np.asarray(w_real) + 1j * np.asarray(w_imag)).reshape(40, 40)

    frs, fis = [], []
    for bb in range(b):
        s = float(P[bb]) ** (1.0 / 3.0)
        for mm in range(m):
            frs.append(_frame(x_real[bb, :, mm]) * s)
            fis.append(_frame(x_imag[bb, :, mm]) * s)
    fr = np.stack(frs, 1)
    fi = np.stack(fis, 1)
    xfv = np.concatenate([(-fi).reshape(F, -1), fr.reshape(F, -1),
                          fi.reshape(F, -1)], axis=1).astype(bfloat16)

    # G folded: 1/cov=1/2, rows split [0:40)/[40:80) for fused overlap-add
    Gh = G * 0.5
    gwv = np.concatenate([Gh.real[0:HOP].T, Gh.real[HOP:F].T,
                          -Gh.imag[0:HOP].T, -Gh.imag[HOP:F].T,
                          Gh.imag[0:HOP].T, Gh.imag[HOP:F].T],
                         axis=1).astype(bfloat16)

    # permutation matrices for rolls r=1..4 (lhsT[g, f] = 1 iff g=(f-r)%80)
    pparts = []
    g = np.arange(F)
    for r in range(1, NJ):
        Pm = np.zeros((F, F), np.float32)
        Pm[(g - r) % F, g] = 1.0
        pparts.append(Pm)
    pwv = np.concatenate(pparts, axis=1).astype(bfloat16)

    in_maps, shards = [], []
    for ci in range(8):
        sc = 5 * ci - 20
        Ws = np.roll(W, sc, axis=0).T
        fwv = np.concatenate([W.real, W.imag, Ws.real, Ws.imag],
                             axis=1).astype(bfloat16)
        mparts = []
        for r in range(NJ):
            Mr, Mi = _m_mats(w2, sc + r)
            mparts += [Mr, -Mi, Mi]
        mwv = np.concatenate(mparts, axis=1).astype(bfloat16)
        in_maps.append({"xf": xfv, "fw": fwv, "pw": pwv, "mw": mwv,
                        "gw": gwv})
        shards.append(ci)

    cov = np.zeros(L)
    idx = (np.arange(T)[:, None] * HOP + np.arange(F)[None, :]).reshape(-1)
    np.add.at(cov, idx, 1.0)
    cov = np.where(cov > 0, cov, 1.0)
    return in_maps, shards, P, cov


_NC_CACHE = {}


def kernel(x_real, x_imag, task_info, w_real, w_imag, b_real, b_imag):
    x_real = np.asarray(x_real)
    x_imag = np.asarray(x_imag)
    task_info = np.asarray(task_info)
    b, Lx, m = x_real.shape
    assert (b, Lx, m) == (2, L, 2)

    if "nc" not in _NC_CACHE:
        nc_ = build_program(debug=False)
        nc_.compile()
        _NC_CACHE["nc"] = nc_
    nc = _NC_CACHE["nc"]

    in_maps, shards, P, cov = make_in_maps(x_real, x_imag, task_info,
                                           w_real, w_imag)
    from concourse.bass_utils import run_bass_kernel_spmd
    res = run_bass_kernel_spmd(nc, in_maps, list(range(8))).results

    CW = BM * TP
    Ysum = np.zeros((HOP, 2 * CW), np.float64)
    for i in range(8):
        Ysum += np.asarray(res[i]["yv"], np.float64)
    Y = Ysum.reshape(HOP, 2, BM, TP)

    x = (x_real + 1j * x_imag).astype(np.complex64)
    out = x.copy()
    bias = complex(np.asarray(b_real)[0], np.asarray(b_imag)[0])
    bias_sig = np.zeros(L, np.complex64)
    bias_sig[np.arange(T) * HOP] = bias
    bias_sig /= cov
    for u in range(BM):
        bb, mm = divmod(u, m)
        yr = Y[:, 0, u].T.ravel()[:L]
        yi = Y[:, 1, u].T.ravel()[:L]
        out[bb, :, mm] += (yr + 1j * yi).astype(np.complex64)
        out[bb, :, mm] += (P[bb] * bias_sig).astype(np.complex64)
    return out[:, 20:L - 20, :]
